# revision 15
# baseline (speedup 1.0000x reference)
"""AncProbsLayer on 8 TRN2 NeuronCores.

tauQ[m,b,k] = mut_rates[m,b,k] * Q[m,k], so P[m,b,k] = expm(tauQ) depends on
only m*k tiny rate matrices and is computed exactly on host (f64). The device
runs the heavy batched einsum out[m,b] = A[m,b] @ concat_k P[m,b,k], i.e.
(1024,20)@(20,80) per (m,b) pair, data-parallel over b on 8 cores.

Device-side encoding: host scales each P column by s = colsum(P)/254 so that
A @ P' lands in [0, 254) (A is uniform [0,1), P > 0). PSUM f32 results are
cast straight to uint8 -- HALF the output DMA bytes of bf16 at bf16-class
accuracy (norm rel err ~2.9e-3; the +-0.5 LSB quantization noise is tiny vs
the column magnitude). Host decodes out = u8 * s.

Per core: 64 (m,b) pairs in 20 groups of 3 + 1 rump group of 4, stacked
block-diagonally (contraction 60/80 partitions, N=240/320 free) -- small
groups keep the block-diag rhs padding low (0.63MB vs 1.15MB at 6-pair
groups) while N=240 matmuls still stream at full PE rate. Four matmuls fill
a 2-bank f32 PSUM tile; one strided 960-col cast (DVE for even items, ACT
for odd) downcasts to the uint8 staging tile. Output DMAs are fine-grained
(1920-col slices issued as soon as their two items are cast) so the HBM
write stream chases the compute instead of piling into a tail. Input a-DMAs
issue from Sync, rhs-DMAs from ScalarE (both HWDGE) so no single issue queue
serializes the start.
"""

import numpy as np
import ml_dtypes

import concourse.bass as bass
import concourse.mybir as mybir
from concourse.tile import TileContext
from concourse.bass_utils import run_bass_kernel_spmd

S = 20          # amino acids
M_ = 2          # models
B = 256         # sequence batch
L = 1024        # sequence length
K = 4           # matrices per model
KS = K * S      # 80 output columns per pair
N_CORES = 8
BS = B // N_CORES          # 32 sequences per core
PAIRS = M_ * BS            # 64 (m,b) pairs per core
CH = L // 128              # 8 row chunks of 128

GP = 6                     # pairs per full group (K=120 keeps the PE HAM warm;
                           # K=61 groups measured stuck at the cold 1.2 GHz clock)
NG = 10                    # full groups
RUMP = PAIRS - GP * NG     # 4 pairs in the rump group
NF = GP * KS               # 480 matmul free cols (full group)
NR = RUMP * KS             # 320 (rump)

# each engine stages 2 rump items (640 cols) then 20 full items (960 cols)
ENG_W = 2 * 2 * NR + NG * 2 * 2 * NF   # 20480 uint8 cols per engine
OUT_W = 2 * ENG_W          # 40960; std -> out[:, :ENG_W], sta -> the rest
FBASE = 2 * 2 * NR         # 1280: full-group items start here in each tile

# quantization: s = colsum(P)/QDEN so A@P' < 254 (A<1, P>0); decode adds QOFF
# (0.0 if the f32->u8 cast rounds to nearest, 0.5 if it floors)
QDEN = 254.0
QOFF = 0.0

BF16 = mybir.dt.bfloat16
NPBF16 = ml_dtypes.bfloat16

TRACE = False
TRACE_DIR = None
LAST = {"exec_time_ns": None}
_NC_CACHE = {}

# input DMA slicing (by full-group index); a-DMAs ride Sync's HWDGE queue,
# r-DMAs GpSimd's SWDGE (ScalarE must spend its cycles on casts only)
A_SLICES = [(0, 1), (1, 2), (2, 4), (4, 7), (7, 10)]
R_SLICES = [(0, 1), (1, 4), (4, 10)]
# output DMA slice widths (uint8 cols) within each engine's staging tile:
# the rump pair first, 3-item slices through the middle, single items at the
# end so the final DMA is small. DVE slices issue from GpSimd, ACT from Sync.
O_WIDTHS = [4 * NR, 3 * 960, 3 * 960, 3 * 960, 3 * 960, 3 * 960, 3 * 960,
            960, 960]
assert sum(O_WIDTHS) == ENG_W


def _install_trace_shims():
    """Test-only: register the NTFF profile hook (missing from this image's
    antenv) and defang the artifact upload so trace=True works locally."""
    import sys as _sys
    import types as _types

    try:
        from antenv.axon_hooks import get_axon_ntff_profile_hook  # noqa: F401
    except ImportError:
        from trn_agent_boot.trn_boot import _ntff_profile_via_ctypes

        hook = _ntff_profile_via_ctypes("/opt/axon/libaxon_pjrt.so")
        mod = _types.ModuleType("antenv.axon_hooks")
        mod.get_axon_ntff_profile_hook = lambda: hook
        mod.set_axon_ntff_profile_hook = lambda h: None
        _sys.modules["antenv.axon_hooks"] = mod

    import concourse.bass_utils as bu

    bu.upload_artifacts = lambda tmpdir: str(tmpdir)


def _split_multi_waits(nc):
    """walrus codegen on this toolchain supports one sync-wait slot per
    instruction; Tile's kernel-tail drain accumulates one wait per touched
    semaphore. Split extra waits onto single-wait NoOps on the same engine."""
    f = nc.m.functions[0]
    for blk in f.blocks:
        insts = blk.instructions
        i = 0
        while i < len(insts):
            inst = insts[i]
            si = getattr(inst, "sync_info", None)
            if si is not None and si.on_wait and len(si.on_wait) > 1:
                waits = list(si.on_wait)
                # The kernel-tail drain waits on every touched semaphore;
                # the output-DMA completions transitively dominate all
                # compute/input-DMA ticks, so keep only those.
                sw = [w for w in waits if "DMASW" in (w.ant_name or "")
                      or "DMAHW" in (w.ant_name or "")]
                if isinstance(inst, mybir.InstDrain) and sw:
                    waits = sw
                elif isinstance(inst, mybir.InstDMACopy):
                    # DMA waits are handled at the issuing sequencer (HWDGE
                    # and SWDGE alike), so extra waits may ride NoOps placed
                    # just before the DMA on its engine. Keep the producer
                    # wait on the DMA itself (the ring handles it); the
                    # lane-reuse guards (DMAHW/DMASW, satisfied much
                    # earlier) go onto the NoOps.
                    waits = sw + [w for w in waits if w not in sw]
                for w in waits[:-1]:
                    nop = mybir.InstNoOp(
                        name=nc.get_next_instruction_name(),
                        sync_info=mybir.SyncInfo(on_wait=[w], on_update=[]),
                        bass_nofuse=True,
                        engine=inst.engine,
                    )
                    nc.register_instruction(nop)
                    insts.insert(i, nop)
                    i += 1
                si.on_wait = [waits[-1]]
            i += 1


def _build_nc():
    if "nc" in _NC_CACHE:
        return _NC_CACHE["nc"]
    nc = bass.Bass()
    am = nc.declare_dram_parameter("am", [GP * S, NG * L], BF16, False)
    a4 = nc.declare_dram_parameter("a4", [RUMP * S, L], BF16, False)
    rm = nc.declare_dram_parameter("rm", [GP * S, NG * NF], BF16, False)
    r4 = nc.declare_dram_parameter("r4", [RUMP * S, NR], BF16, False)
    out = nc.declare_dram_parameter("out", [128, OUT_W], mybir.dt.uint8, True)

    with TileContext(nc) as tc:
        with (
            tc.tile_pool(name="ins", bufs=1) as ins,
            tc.tile_pool(name="st", bufs=1) as stp,
            tc.tile_pool(name="ps", bufs=2, space="PSUM") as ps,
        ):
            am_t = ins.tile([GP * S, NG * L], BF16, tag="am", name="am")
            a4_t = ins.tile([RUMP * S, L], BF16, tag="a4", name="a4")
            rm_t = ins.tile([GP * S, NG * NF], BF16, tag="rm", name="rm")
            r4_t = ins.tile([RUMP * S, NR], BF16, tag="r4", name="r4")

            # the rump group runs first, so its (small) inputs load first;
            # a-loads on Sync's HWDGE queue, r-loads on GpSimd's SWDGE
            nc.gpsimd.dma_start(out=r4_t[:], in_=r4[:])
            nc.sync.dma_start(out=a4_t[:], in_=a4[:])
            for g0, g1 in R_SLICES:
                nc.gpsimd.dma_start(out=rm_t[:, g0 * NF:g1 * NF],
                                    in_=rm[:, g0 * NF:g1 * NF])
            for g0, g1 in A_SLICES:
                nc.sync.dma_start(out=am_t[:, g0 * L:g1 * L],
                                  in_=am[:, g0 * L:g1 * L])

            std = stp.tile([128, ENG_W], mybir.dt.uint8, tag="std", name="std")
            sta = stp.tile([128, ENG_W], mybir.dt.uint8, tag="sta", name="sta")

            # per-engine output-DMA pacing state: (staged cols, next slice
            # index, cols already shipped)
            state = {"d": [0, 0, 0], "a": [0, 0, 0]}

            def emit(eng, cols):
                st_tile = std if eng == "d" else sta
                base = 0 if eng == "d" else ENG_W
                s = state[eng]
                s[0] += cols
                while s[1] < len(O_WIDTHS) and \
                        s[0] >= s[2] + O_WIDTHS[s[1]]:
                    b0, b1 = s[2], s[2] + O_WIDTHS[s[1]]
                    if eng == "d":
                        nc.gpsimd.dma_start(out=out[:, base + b0:base + b1],
                                            in_=st_tile[:, b0:b1])
                    else:
                        nc.sync.dma_start(out=out[:, base + b0:base + b1],
                                          in_=st_tile[:, b0:b1])
                    s[2] = b1
                    s[1] += 1

            def item(lhs_t, rhs_ap, lbase, n, eng, col):
                """2 matmuls into one 2-bank psum tile + 1 strided cast."""
                pt = ps.tile([128, 1024], mybir.dt.float32,
                             tag="psD" if eng == "d" else "psA", bufs=2)
                for h in (0, 1):
                    nc.tensor.matmul(
                        pt[:, h * 512:h * 512 + n],
                        lhs_t[:, lbase + h * 128:lbase + h * 128 + 128],
                        rhs_ap, start=True, stop=True)
                src = pt.rearrange("p (h x) -> p h x", h=2)[:, :, :n]
                st_tile = std if eng == "d" else sta
                dst = st_tile[:, col:col + 2 * n].rearrange(
                    "p (h x) -> p h x", h=2)
                if eng == "d":
                    nc.vector.tensor_copy(out=dst, in_=src)
                else:
                    nc.scalar.copy(out=dst, in_=src)
                emit(eng, 2 * n)

            # rump first: items e=0..3 cover chunks (2e, 2e+1), DVE/ACT alt.
            for e in range(4):
                item(a4_t, r4_t[:], 2 * e * 128, NR,
                     "d" if e % 2 == 0 else "a", (e // 2) * 2 * NR)
            # full groups: item (g,d) covers chunks (2d, 2d+1)
            for g in range(NG):
                rhs = rm_t[:, g * NF:(g + 1) * NF]
                for d in range(4):
                    k = 2 * g + d // 2
                    item(am_t, rhs, g * L + 2 * d * 128, NF,
                         "d" if d % 2 == 0 else "a", FBASE + k * 960)
    _split_multi_waits(nc)
    _NC_CACHE["nc"] = nc
    return nc


def _softplus(x):
    return np.logaddexp(0.0, x)


def _host_pcat(tau_kernel, exchangeability_kernel, equilibrium_kernel,
               per_matrix_rates_kernel, rate_indices):
    """(m,b,S,K*S) float64: per-(m,b) transition matrices, concatenated over k."""
    tk = np.asarray(tau_kernel, dtype=np.float64)
    ek = np.asarray(exchangeability_kernel, dtype=np.float64)
    qk = np.asarray(equilibrium_kernel, dtype=np.float64)
    pk = np.asarray(per_matrix_rates_kernel, dtype=np.float64)
    idx = np.asarray(rate_indices, dtype=np.int64)

    tau = _softplus(np.take_along_axis(tk, idx, axis=1))           # (m,b)
    pmr = _softplus(pk)                                            # (m,k)
    mut = tau[:, :, None] * pmr[:, None, :]                        # (m,b,k)

    R = _softplus(0.5 * (ek + np.swapaxes(ek, -1, -2)))
    R = R * (1.0 - np.eye(S))                                      # (m,k,S,S)
    e = qk - qk.max(axis=-1, keepdims=True)
    p = np.exp(e)
    p /= p.sum(axis=-1, keepdims=True)                             # (m,k,S)

    Q = R * p[:, :, None, :]
    diag = Q.sum(axis=-1, keepdims=True)                           # (m,k,S,1)
    Q = Q - diag * np.eye(S)
    mue = np.sum(p[..., None] * diag, axis=-2, keepdims=True)      # (m,k,1,1)
    Q = Q / np.maximum(mue, 1e-16)

    A = mut[..., None, None] * Q[:, None]                          # (m,b,k,S,S)
    A = A / 64.0                                                   # 2^-6 scaling
    eye = np.broadcast_to(np.eye(S), A.shape)
    out = eye.copy()
    term = eye.copy()
    for i in range(1, 15):
        term = term @ A / i
        out = out + term
    for _ in range(6):
        out = out @ out
    # (m,b,k,z,s) -> (m,b,z,k*s)
    return out.transpose(0, 1, 3, 2, 4).reshape(M_, B, S, KS)


def kernel(inputs, tau_kernel, exchangeability_kernel, equilibrium_kernel,
           per_matrix_rates_kernel, rate_indices):
    inputs = np.asarray(inputs)
    pcat = _host_pcat(tau_kernel, exchangeability_kernel, equilibrium_kernel,
                      per_matrix_rates_kernel, rate_indices)   # (m,B,S,KS) f64
    s = pcat.sum(axis=2) / QDEN                                # (m,B,KS)
    ps_scaled = (pcat / s[:, :, None, :]).astype(np.float32)   # (m,B,S,KS)

    in_maps = []
    for core in range(N_CORES):
        bsl = slice(core * BS, (core + 1) * BS)
        ap = inputs[:, bsl].reshape(PAIRS, L, S)               # (64,L,S) f32
        # am[j*S+z, g*L+l] = A[3g+j, l, z]
        am = np.ascontiguousarray(
            ap[:GP * NG].reshape(NG, GP, L, S).transpose(1, 3, 0, 2)
        ).reshape(GP * S, NG * L).astype(NPBF16)
        a4 = np.ascontiguousarray(
            ap[GP * NG:].transpose(0, 2, 1)).reshape(RUMP * S, L).astype(NPBF16)
        pc = ps_scaled[:, bsl].reshape(PAIRS, S, KS)           # (64,S,80)
        rm = np.zeros((GP * S, NG, NF), np.float32)
        pg = pc[:GP * NG].reshape(NG, GP, S, KS)
        for j in range(GP):
            rm[j * S:(j + 1) * S, :, j * KS:(j + 1) * KS] = \
                pg[:, j].transpose(1, 0, 2)
        rm = rm.reshape(GP * S, NG * NF)
        r4 = np.zeros((RUMP * S, NR), np.float32)
        for j in range(RUMP):
            r4[j * S:(j + 1) * S, j * KS:(j + 1) * KS] = pc[GP * NG + j]
        in_maps.append({
            "am": am, "a4": a4,
            "rm": rm.astype(NPBF16), "r4": r4.astype(NPBF16),
        })

    nc = _build_nc()
    if TRACE:
        _install_trace_shims()
        res = run_bass_kernel_spmd(nc, in_maps, list(range(N_CORES)),
                                   trace=True, tmpdir=TRACE_DIR)
    else:
        res = run_bass_kernel_spmd(nc, in_maps, list(range(N_CORES)))
    LAST["exec_time_ns"] = res.exec_time_ns

    sf = s.astype(np.float32)                                  # (m,B,KS)
    full = np.empty((M_, B, L, KS), np.float32)
    for core in range(N_CORES):
        bsl = slice(core * BS, (core + 1) * BS)
        r = np.asarray(res.results[core]["out"])               # (128,OUT_W) u8
        y = np.empty((PAIRS, L, KS), np.float32)
        # engine tile layout: [2 rump items of 640][20 full items of 960].
        # DVE (first half of out) holds chunks {4e, 4e+1}, ACT {4e+2, 4e+3};
        # within an item cols are h*width + j*KS + t, chunk c = 4e(+2)+h,
        # l = c*128 + p
        for ei, eng_cols in enumerate((r[:, :ENG_W], r[:, ENG_W:])):
            lb = 256 * ei                     # ACT chunks sit 2*128 later
            arr = eng_cols[:, :FBASE].reshape(128, 2, 2, RUMP, KS)
            yr = np.transpose(arr, (3, 1, 2, 0, 4)).reshape(RUMP, 2, 256, KS)
            y[GP * NG:, lb:lb + 256] = yr[:, 0]
            y[GP * NG:, 512 + lb:512 + lb + 256] = yr[:, 1]
            arr = eng_cols[:, FBASE:].reshape(128, NG, 2, 2, GP, KS)
            yf = np.transpose(arr, (1, 4, 2, 3, 0, 5)).reshape(
                GP * NG, 2, 256, KS)
            y[:GP * NG, lb:lb + 256] = yf[:, 0]
            y[:GP * NG, 512 + lb:512 + lb + 256] = yf[:, 1]
        if QOFF:
            y += QOFF
        y *= sf[:, bsl].reshape(PAIRS, 1, KS)
        full[:, bsl] = y.reshape(M_, BS, L, KS)
    return full
